# revision 23
# baseline (speedup 1.0000x reference)
"""Trainium2 Bass kernel for nn_ConvDiscriminator (ragged CNN discriminator).

Math (per sample b with length L):
  flat = encoder_output[0:L, b, :].ravel()           # contiguous [L*512]
  X[h, l] = flat[h*L + l]  (raw reshape to [512, L], zero-pad cols >= L)
  conv_w (w=1..5): out_w[f, t] = sum_{h,dw} Ww[f,h,dw] * X[h, t+dw]
  pool_w[f] = relu(bias_w[f] + max_{t <= Leff-w} out_w[f, t])
  fc1 -> fc2 -> sigmoid

Kernel strategy (8 cores, uniform SPMD program, per-core data tables):
  - Sort the 128 samples by length desc; slot j holds ranks [8j, 8j+8), one
    per core.  Canonical slot width Wc[j] = max length in slot; slots are
    bin-packed (first-fit decreasing) into "packs" of total width <= 512 so
    each (pack, w) is one PSUM bank and the conv matmuls stay wide (the
    ~85ns LDWEIGHTS per matmul hides under the column stream).
  - The ragged raw-reshape is done on HOST: per pack the [128, 4*Wpad] tile
    F[p, k*Wpad + off_j + t] = flat_j[(4p+k)*L + t] (zero pad elsewhere) is
    materialized in fp8 and DMA'd to SBUF as a plain strided copy, spread
    over both HWDGE rings (sync + scalar) since each dma_start costs ~650ns
    of sequencer issue time.
  - fp8e4m3 DoubleRow matmuls (2 k-pair steps over H=512); conv weights are
    pre-scaled by WSCALE=2^8 on host so sigma=0.02 values land in e4m3
    normal range; the dequant folds into host-side fc1 weight scaling.
  - Validity masking: after each (pack, w) accumulation group closes,
    narrow gpsimd tensor_tensor adds of host -1e30 rows cover the per-slot
    tail bands (sample tails per that core's length, boundary-crossing and
    padded columns for every core); then per-slot vector reduce_max.
  - The last pack runs w=5..1 with the fc1 accumulation interleaved so the
    fc chain hides inside the conv stream; final sigmoid on host.
"""

import os
import sys

for _p in ("/opt/trn_rl_repo", "/root/.axon_site/_ro/trn_rl_repo"):
    if os.path.isdir(_p) and _p not in sys.path:
        sys.path.insert(0, _p)

import numpy as np
import ml_dtypes

T = 512
B = 128
H = 512
NF = 128
FS = 5
P = 128
NCORES = 8
NSLOT = B // NCORES  # 16

USE_FP8 = True  # fp8e4m3 DoubleRow conv matmuls (weights pre-scaled by WSCALE)
WSCALE = 256.0
MASK_ENGINE = "vector"  # gpsimd cannot access PSUM on TRN2

LAST_EXEC_NS = None
LAST_RESULTS = None
_PROGRAM_CACHE = {}

# wconv tile split: tile name -> (first pair_index, n k-blocks)
_WTILES = {1: ("w1", 0, 4), 2: ("w2", 4, 8), 3: ("w3", 12, 12),
           4: ("w45", 24, 36), 5: ("w45", 24, 36)}
# local k-block base of each w within its tile
_WBASE = {1: 0, 2: 0, 3: 0, 4: 0, 5: 16}


def _pair_index(w, dw):
    # enumerate (w, dw) pairs: w=1..5, dw=0..w-1 -> 0..14
    return (w - 1) * w // 2 + dw


def _pad16(x):
    return -(-x // 16) * 16


def make_packs(Wc, Lmin):
    """First-fit-decreasing bin pack of slots into <=512-col PSUM groups.

    Returns list of packs: dict(Wsum, Wpad, WB, slots=[(j, off, Wcj, Lminj)]).
    """
    order = sorted(range(NSLOT), key=lambda j: -Wc[j])
    packs = []
    for j in order:
        placed = False
        for pk in packs:
            if _pad16(pk["w"] + Wc[j]) <= 512:
                pk["slots"].append(j)
                pk["w"] += Wc[j]
                placed = True
                break
        if not placed:
            packs.append({"w": Wc[j], "slots": [j]})
    # process reduce-heavy (multi-slot) packs first so the vector engine's
    # reduce backlog drains before the stream tail; a single-slot pack last
    packs.sort(key=lambda pk: (-len(pk["slots"]), -pk["w"]))
    out = []
    for pk in packs:
        offs = []
        o = 0
        for j in pk["slots"]:
            offs.append((j, o, Wc[j], Lmin[j]))
            o += Wc[j]
        out.append({"Wsum": o, "Wpad": _pad16(o), "WB": _pad16(o) + 8, "slots": offs})
    return out


def build_program(packs_key, use_fp8=True):
    import concourse.bass as bass
    import concourse.bacc as bacc
    import concourse.mybir as mybir
    from concourse.tile import TileContext

    f32 = mybir.dt.float32
    cdt = mybir.dt.bfloat16
    wdt = mybir.dt.float8e4 if use_fp8 else cdt  # conv weights + F tiles
    AX = mybir.AxisListType
    ADD = mybir.AluOpType.add

    packs = [
        {"Wsum": Wsum, "Wpad": Wpad, "WB": WB, "slots": list(slots)}
        for (Wsum, Wpad, WB, slots) in packs_key
    ]
    npk = len(packs)
    Stot = sum(4 * pk["Wpad"] for pk in packs)
    foff = []
    o = 0
    for pk in packs:
        foff.append(o)
        o += 4 * pk["Wpad"]
    mo = []
    o = 0
    for pk in packs:
        mo.append(o)
        o += pk["WB"]
    ONESOFF = o
    MTOT = o + P  # trailing 128 cols of 1.0: K=1 lhsT for tensor-side masks
    WB0 = packs[0]["WB"]

    nc = bacc.Bacc()
    encF = nc.declare_dram_parameter("encF", [P, Stot], wdt, isOutput=False)
    # -1e30 at invalid mask positions, replicated on all 128 partitions
    msk = nc.declare_dram_parameter("msk", [P, MTOT], cdt, isOutput=False)
    wconv = nc.declare_dram_parameter("wconv", [P, 60 * P], wdt, isOutput=False)
    # cbias*WSCALE [:, :5] ++ fc1b (col 5) in one f32 tensor
    fcon = nc.declare_dram_parameter("fcon", [P, 7], f32, isOutput=False)
    # fc1w/WSCALE tiles ++ fc2w (col 500) in one bf16 tensor
    fcw = nc.declare_dram_parameter("fcw", [P, 5 * 100 + 1], cdt, isOutput=False)
    out = nc.declare_dram_parameter("out", [1, NSLOT], f32, isOutput=True)

    with TileContext(nc) as tc:
        with (
            tc.tile_pool(name="const", bufs=1) as constp,
            tc.tile_pool(name="pspool", bufs=8, space="PSUM") as pspool,
        ):
            wsbs = {
                name: constp.tile([P, nb * P], wdt, tag=name, name=name)
                for name, (_, i0, nb) in {v[0]: (0, v[1], v[2]) for v in _WTILES.values()}.items()
            }
            fts = [
                constp.tile([P, 4 * pk["Wpad"]], wdt, tag=f"ft{pi}", name=f"ft{pi}")
                for pi, pk in enumerate(packs)
            ]
            mska = constp.tile([P, WB0], cdt, tag="mska", name="mska")
            mskb = (
                constp.tile([P, MTOT - WB0], cdt, tag="mskb", name="mskb")
                if MTOT > WB0
                else None
            )
            fcon_sb = constp.tile([P, 7], f32, tag="fcon", name="fcon")
            fcw_sb = constp.tile([P, 5 * 100 + 1], cdt, tag="fcw", name="fcw")

            def load_wt(name):
                _, i0, nb = next(v for v in _WTILES.values() if v[0] == name)
                nc.sync.dma_start(
                    out=wsbs[name][:], in_=wconv[:, i0 * P : (i0 + nb) * P]
                )

            def load_f(eng, pi):
                eng.dma_start(
                    out=fts[pi][:],
                    in_=encF[:, foff[pi] : foff[pi] + 4 * packs[pi]["Wpad"]],
                )

            def load_wt_s(name):
                _, i0, nb = next(v for v in _WTILES.values() if v[0] == name)
                nc.scalar.dma_start(
                    out=wsbs[name][:], in_=wconv[:, i0 * P : (i0 + nb) * P]
                )

            # sync (SP) ring: first pack + w1/w45 weights, then odd packs
            load_wt("w1")
            if npk > 0:
                load_f(nc.sync, 0)
            load_wt("w45")
            for pi in range(1, npk, 2):
                load_f(nc.sync, pi)
            # scalar (ACT) ring: early w2/w3 weights + pack-0 mask, mask blob
            load_wt_s("w2")
            nc.scalar.dma_start(out=mska[:], in_=msk[:, 0:WB0])
            load_wt_s("w3")
            if npk > 2:
                load_f(nc.scalar, 2)
            if mskb is not None:
                nc.scalar.dma_start(out=mskb[:], in_=msk[:, WB0:MTOT])
            for pi in range(4, npk, 2):
                load_f(nc.scalar, pi)
            nc.scalar.dma_start(out=fcon_sb[:], in_=fcon[:])
            nc.scalar.dma_start(out=fcw_sb[:], in_=fcw[:])

            cb_sb = fcon_sb[:, 0:FS]
            fc1b_sb = fcon_sb[:100, FS : FS + 1]
            fc1w_sb = fcw_sb[:, 0 : 5 * 100]
            fc2w_sb = fcw_sb[:100, 5 * 100 : 5 * 100 + 1]

            pools = [
                constp.tile([P, NSLOT], f32, tag=f"pool{w}", name=f"pool{w}")
                for w in range(1, FS + 1)
            ]
            poolsr = [
                constp.tile([P, NSLOT], cdt, tag=f"poolr{w}", name=f"poolr{w}")
                for w in range(1, FS + 1)
            ]

            mask_eng = nc.gpsimd if MASK_ENGINE == "gpsimd" else nc.vector
            psf1 = pspool.tile([100, NSLOT], f32, tag="ps", name="psf1")

            def conv_w(pi, w):
                pk = packs[pi]
                Wsum, Wpad = pk["Wsum"], pk["Wpad"]
                nslots = len(pk["slots"])
                Npack = Wsum - w + 1
                ps = pspool.tile([P, Npack], f32, tag="ps", name=f"ps{pi}w{w}")
                tname = _WTILES[w][0]
                wt = wsbs[tname]
                nb = _WTILES[w][2]
                if use_fp8:
                    pairs = [(dw, k0) for dw in range(w) for k0 in (0, 2)]
                else:
                    pairs = [(dw, k) for dw in range(w) for k in range(4)]
                # bf16 K=1 matmuls inside fp8 DoubleRow groups force PE mode
                # switches (~4.5us total measured) — keep masks on vector
                tmask = False
                mt, mofs = (mska, mo[pi]) if pi == 0 else (mskb, mo[pi] - WB0)
                ones_lhsT = (
                    mskb[:1, ONESOFF - WB0 : ONESOFF - WB0 + P]
                    if mskb is not None
                    else None
                )

                def bands():
                    for si, (j, off, Wcj, Lmj) in enumerate(pk["slots"]):
                        b0 = off + max(0, min(Lmj - w + 1, Wcj))
                        b1 = off + Wcj if si < nslots - 1 else Npack
                        if b0 < b1:
                            yield b0, b1

                for n, (dw, k) in enumerate(pairs):
                    idx = _WBASE[w] + dw * 4 + k
                    if use_fp8:
                        nc.tensor.matmul(
                            ps[:],
                            wt[:].rearrange("p (k m) -> p k m", k=nb)[
                                :, idx : idx + 2, :
                            ],
                            fts[pi][:].rearrange("p (k w) -> p k w", k=4)[
                                :, k : k + 2, dw : dw + Npack
                            ],
                            start=(n == 0),
                            stop=(n == len(pairs) - 1),
                            perf_mode=mybir.MatmulPerfMode.DoubleRow,
                        )
                        if n == 0 and tmask:
                            for b0, b1 in bands():
                                nc.tensor.matmul(
                                    ps[:, b0:b1],
                                    ones_lhsT,
                                    mt[:1, mofs + b0 + w : mofs + b1 + w],
                                    start=False,
                                    stop=False,
                                    tile_position=(0, 0),
                                )
                    else:
                        nc.tensor.matmul(
                            ps[:],
                            wt[:, idx * P : (idx + 1) * P],
                            fts[pi][:, k * Wpad + dw : k * Wpad + dw + Npack],
                            start=(n == 0),
                            stop=(n == len(pairs) - 1),
                        )
                if not tmask:
                    for b0, b1 in bands():
                        mask_eng.tensor_tensor(
                            ps[:, b0:b1],
                            ps[:, b0:b1],
                            mt[:, mofs + b0 + w : mofs + b1 + w],
                            ADD,
                        )
                for j, off, Wcj, Lmj in pk["slots"]:
                    nc.vector.reduce_max(
                        pools[w - 1][:, j : j + 1],
                        ps[:, off : off + Wcj - w + 1],
                        axis=AX.X,
                    )

            for pi in range(npk - 1):
                for w in range(1, FS + 1):
                    conv_w(pi, w)
            # last pack: w descending, fc1 accumulation interleaved
            def pool_ts(w):
                # gpsimd TS measured ~480ns vs vector ~180: keep on vector,
                # one engine fewer in the fc dependency chain
                nc.vector.tensor_scalar(
                    poolsr[w - 1][:],
                    pools[w - 1][:],
                    cb_sb[:, w - 1 : w],
                    0.0,
                    mybir.AluOpType.add,
                    mybir.AluOpType.max,
                )

            def fc1_mm(w, first, last):
                nc.tensor.matmul(
                    psf1[:],
                    fc1w_sb[:, (w - 1) * 100 : w * 100],
                    poolsr[w - 1][:],
                    start=first,
                    stop=last,
                )

            # last pack runs w ASCENDING (conv blocks grow toward the end) and
            # each fc1 matmul is emitted two blocks late, so the mask+reduce+
            # TS chain of its w hides under larger and larger conv streams
            for w in range(1, FS + 1):
                conv_w(npk - 1, w)
                pool_ts(w)
                if w - 2 >= 1:
                    fc1_mm(w - 2, first=(w - 2 == 1), last=False)
            fc1_mm(FS - 1, first=False, last=False)
            fc1_mm(FS, first=False, last=True)

            fc1_sb = constp.tile([100, NSLOT], cdt, tag="fc1o")
            nc.vector.tensor_scalar(
                fc1_sb[:], psf1[:], fc1b_sb, None, mybir.AluOpType.add
            )
            psf2 = pspool.tile([1, NSLOT], f32, tag="ps", name="psf2")
            nc.tensor.matmul(psf2[:], fc2w_sb, fc1_sb[:], start=True, stop=True)
            out_sb = constp.tile([1, NSLOT], f32, tag="outsb")
            nc.vector.tensor_scalar(
                out_sb[:], psf2[:], 0.0, None, mybir.AluOpType.add
            )
            nc.sync.dma_start(out=out[:], in_=out_sb[:])

    nc.compile()
    return nc


def prepare(encoder_output, lengths, conv_ws, conv_bs, fc1_w, fc1_b, fc2_w, fc2_b,
            use_fp8=None):
    """Host-side prep: sample assignment, per-core data tables, program build.

    Returns (nc, in_maps, assignment, fc2b) where assignment[c][j] = sample.
    """
    if use_fp8 is None:
        use_fp8 = USE_FP8
    enc = np.ascontiguousarray(np.asarray(encoder_output, dtype=np.float32))
    lens = np.asarray(lengths).astype(np.int64)
    assert enc.shape == (T, B, H)
    assert lens.shape == (B,)

    cdt = ml_dtypes.bfloat16
    wdt = ml_dtypes.float8_e4m3 if use_fp8 else cdt
    wscale = np.float32(WSCALE if use_fp8 else 1.0)

    # effective lengths (torch zero-pads width to >= filter_size)
    eff = np.maximum(lens, FS)

    # sort desc by effective length; slot j <- ranks [8j, 8j+8)
    ranks = np.argsort(-eff, kind="stable")
    assignment = [[int(ranks[8 * j + c]) for j in range(NSLOT)] for c in range(NCORES)]
    Wc = tuple(int(eff[ranks[8 * j]]) for j in range(NSLOT))
    Lmin = tuple(int(eff[ranks[8 * j + NCORES - 1]]) for j in range(NSLOT))
    packs = make_packs(Wc, Lmin)
    packs_key = tuple(
        (pk["Wsum"], pk["Wpad"], pk["WB"], tuple(pk["slots"])) for pk in packs
    )

    encT = enc.transpose(1, 0, 2)  # [B, T, H], sample-major views

    Stot = sum(4 * pk["Wpad"] for pk in packs)
    foff = []
    o = 0
    for pk in packs:
        foff.append(o)
        o += 4 * pk["Wpad"]
    mo = []
    o = 0
    for pk in packs:
        mo.append(o)
        o += pk["WB"]
    ONESOFF = o
    MTOT = o + P

    in_maps = []
    for c in range(NCORES):
        encF_c = np.zeros((P, Stot), dtype=np.float32)
        mrow_all = np.zeros(MTOT, dtype=np.float32)  # [.., ONESOFF): masks; rest: 1.0
        for pi, pk in enumerate(packs):
            Wpad = pk["Wpad"]
            blk = encF_c[:, foff[pi] : foff[pi] + 4 * Wpad].reshape(P, 4, Wpad)
            mrow = mrow_all[mo[pi] : mo[pi] + pk["WB"]]
            nslots = len(pk["slots"])
            Les = []
            for si, (j, off, Wcj, Lmj) in enumerate(pk["slots"]):
                b = assignment[c][j]
                L = int(lens[b])
                Le = int(eff[b])
                Les.append(Le)
                flat = encT[b].reshape(-1)[: H * L]
                blk[:, :, off : off + L] = flat.reshape(P, 4, L)
                # mask row: 1 (invalid) where u > off + Le, u in this slot's
                # span [off, off+Wcj) (last slot: through WB)
                hi = off + Wcj if si < nslots - 1 else pk["WB"]
                u = np.arange(off, hi)
                mrow[off:hi] = (u > off + Le).astype(np.float32)
            for si, (j, off, Wcj, Lmj) in enumerate(pk["slots"]):
                if si < nslots - 1:
                    # u in [off+Wcj, off+Wcj+FS) is read only by THIS slot's
                    # bands: u == off+Wcj iff this core's sample is short;
                    # u > off+Wcj is a boundary-crossing window, always bad
                    mrow[off + Wcj] = 1.0 if Les[si] < Wcj else 0.0
                    mrow[off + Wcj + 1 : min(off + Wcj + FS, pk["WB"])] = 1.0
        mrow_all *= np.float32(-1e30)
        mrow_all[ONESOFF:] = 1.0
        in_maps.append(
            {
                "encF": encF_c.astype(wdt),
                "msk": np.broadcast_to(
                    mrow_all.astype(cdt)[None, :], (P, MTOT)
                ).copy(),
            }
        )

    # weights, shared across cores
    wconv = np.empty((P, 60 * P), dtype=np.float32)
    hsel = np.arange(P)[:, None] * 4  # [128,1]
    for w in range(1, FS + 1):
        Ww = np.asarray(conv_ws[w - 1], dtype=np.float32)  # [NF, 1, H, w]
        for dw in range(w):
            i = _pair_index(w, dw)
            for k in range(4):
                # lhsT[p, f] = Ww[f, 0, 4p+k, dw] * wscale
                wconv[:, (i * 4 + k) * P : (i * 4 + k + 1) * P] = (
                    Ww[:, 0, (hsel + k).ravel(), dw].T * wscale
                )
    fcon = np.zeros((P, 7), dtype=np.float32)
    fcon[:, 0:FS] = (
        np.stack([np.asarray(b, dtype=np.float32) for b in conv_bs], axis=1) * wscale
    )
    fcon[:100, FS] = np.asarray(fc1_b, dtype=np.float32)
    fcw_host = np.zeros((P, 5 * 100 + 1), dtype=np.float32)
    fc1_w = np.asarray(fc1_w, dtype=np.float32) / wscale  # [100, 640], dequant
    for k in range(5):
        fcw_host[:, k * 100 : (k + 1) * 100] = fc1_w[:, k * P : (k + 1) * P].T
    fcw_host[:100, 5 * 100] = np.asarray(fc2_w, dtype=np.float32).reshape(-1)
    shared = {
        "wconv": wconv.astype(wdt),
        "fcon": fcon,
        "fcw": fcw_host.astype(cdt),
    }
    for m in in_maps:
        m.update(shared)

    key = (packs_key, use_fp8, MASK_ENGINE)
    if key not in _PROGRAM_CACHE:
        _PROGRAM_CACHE[key] = build_program(packs_key, use_fp8)
    nc = _PROGRAM_CACHE[key]
    fc2b = float(np.asarray(fc2_b, dtype=np.float32).reshape(-1)[0])
    return nc, in_maps, assignment, fc2b


def _ensure_ntff_hook():
    """Install the axon NTFF profile hook if the image's antenv lacks it."""
    import types

    try:
        from antenv.axon_hooks import get_axon_ntff_profile_hook  # noqa: F401
        return True
    except ImportError:
        pass
    try:
        import antenv
        from trn_agent_boot.trn_boot import _ntff_profile_via_ctypes

        hook = _ntff_profile_via_ctypes("/opt/axon/libaxon_pjrt.so")
        mod = types.ModuleType("antenv.axon_hooks")
        _state = {"hook": hook}
        mod.get_axon_ntff_profile_hook = lambda: _state["hook"]
        mod.set_axon_ntff_profile_hook = lambda h: _state.update(hook=h)
        sys.modules["antenv.axon_hooks"] = mod
        antenv.axon_hooks = mod
        return hook is not None
    except Exception as e:  # pragma: no cover
        print(f"ntff hook install failed: {e}", file=sys.stderr)
        return False


def kernel(encoder_output, lengths,
           conv_w1, conv_b1, conv_w2, conv_b2, conv_w3, conv_b3,
           conv_w4, conv_b4, conv_w5, conv_b5,
           fc1_w, fc1_b, fc2_w, fc2_b):
    global LAST_EXEC_NS, LAST_RESULTS
    from concourse.bass_utils import run_bass_kernel_spmd

    conv_ws = [conv_w1, conv_w2, conv_w3, conv_w4, conv_w5]
    conv_bs = [conv_b1, conv_b2, conv_b3, conv_b4, conv_b5]
    nc, in_maps, assignment, fc2b = prepare(
        encoder_output, lengths, conv_ws, conv_bs, fc1_w, fc1_b, fc2_w, fc2_b
    )

    trace = bool(int(os.environ.get("KERNEL_TRACE", "0")))
    if trace:
        trace = _ensure_ntff_hook()
    res = run_bass_kernel_spmd(nc, in_maps, list(range(NCORES)), trace=trace)
    LAST_RESULTS = res
    LAST_EXEC_NS = getattr(res, "exec_time_ns", None)

    out_full = np.empty((B, 1, 1), dtype=np.float32)
    for c in range(NCORES):
        logits = np.asarray(res.results[c]["out"]).reshape(NSLOT).astype(np.float64)
        probs = 1.0 / (1.0 + np.exp(-(logits + fc2b)))
        for j in range(NSLOT):
            out_full[assignment[c][j], 0, 0] = np.float32(probs[j])
    return out_full


# revision 24
# speedup vs baseline: 1.0019x; 1.0019x over previous
"""Trainium2 Bass kernel for nn_ConvDiscriminator (ragged CNN discriminator).

Math (per sample b with length L):
  flat = encoder_output[0:L, b, :].ravel()           # contiguous [L*512]
  X[h, l] = flat[h*L + l]  (raw reshape to [512, L], zero-pad cols >= L)
  conv_w (w=1..5): out_w[f, t] = sum_{h,dw} Ww[f,h,dw] * X[h, t+dw]
  pool_w[f] = relu(bias_w[f] + max_{t <= Leff-w} out_w[f, t])
  fc1 -> fc2 -> sigmoid

Kernel strategy (8 cores, uniform SPMD program, per-core data tables):
  - Sort the 128 samples by length desc; slot j holds ranks [8j, 8j+8), one
    per core.  Canonical slot width Wc[j] = max length in slot; slots are
    bin-packed (first-fit decreasing) into "packs" of total width <= 512 so
    each (pack, w) is one PSUM bank and the conv matmuls stay wide (the
    ~85ns LDWEIGHTS per matmul hides under the column stream).
  - The ragged raw-reshape is done on HOST: per pack the [128, 4*Wpad] tile
    F[p, k*Wpad + off_j + t] = flat_j[(4p+k)*L + t] (zero pad elsewhere) is
    materialized in fp8 and DMA'd to SBUF as a plain strided copy, spread
    over both HWDGE rings (sync + scalar) since each dma_start costs ~650ns
    of sequencer issue time.
  - fp8e4m3 DoubleRow matmuls (2 k-pair steps over H=512); conv weights are
    pre-scaled by WSCALE=2^8 on host so sigma=0.02 values land in e4m3
    normal range; the dequant folds into host-side fc1 weight scaling.
  - Validity masking: after each (pack, w) accumulation group closes,
    narrow gpsimd tensor_tensor adds of host -1e30 rows cover the per-slot
    tail bands (sample tails per that core's length, boundary-crossing and
    padded columns for every core); then per-slot vector reduce_max.
  - The last pack runs w=5..1 with the fc1 accumulation interleaved so the
    fc chain hides inside the conv stream; final sigmoid on host.
"""

import os
import sys

for _p in ("/opt/trn_rl_repo", "/root/.axon_site/_ro/trn_rl_repo"):
    if os.path.isdir(_p) and _p not in sys.path:
        sys.path.insert(0, _p)

import numpy as np
import ml_dtypes

T = 512
B = 128
H = 512
NF = 128
FS = 5
P = 128
NCORES = 8
NSLOT = B // NCORES  # 16

USE_FP8 = True  # fp8e4m3 DoubleRow conv matmuls (weights pre-scaled by WSCALE)
WSCALE = 256.0
MASK_ENGINE = "vector"  # gpsimd cannot access PSUM on TRN2

LAST_EXEC_NS = None
LAST_RESULTS = None
_PROGRAM_CACHE = {}

# wconv tile split: tile name -> (first pair_index, n k-blocks)
_WTILES = {1: ("w1", 0, 4), 2: ("w2", 4, 8), 3: ("w3", 12, 12),
           4: ("w45", 24, 36), 5: ("w45", 24, 36)}
# local k-block base of each w within its tile
_WBASE = {1: 0, 2: 0, 3: 0, 4: 0, 5: 16}


def _pair_index(w, dw):
    # enumerate (w, dw) pairs: w=1..5, dw=0..w-1 -> 0..14
    return (w - 1) * w // 2 + dw


def _pad16(x):
    return -(-x // 16) * 16


def make_packs(Wc, Lmin):
    """First-fit-decreasing bin pack of slots into <=512-col PSUM groups.

    Returns list of packs: dict(Wsum, Wpad, WB, slots=[(j, off, Wcj, Lminj)]).
    """
    order = sorted(range(NSLOT), key=lambda j: -Wc[j])
    packs = []
    for j in order:
        placed = False
        for pk in packs:
            if _pad16(pk["w"] + Wc[j]) <= 512:
                pk["slots"].append(j)
                pk["w"] += Wc[j]
                placed = True
                break
        if not placed:
            packs.append({"w": Wc[j], "slots": [j]})
    # process reduce-heavy (multi-slot) packs first so the vector engine's
    # reduce backlog drains before the stream tail; a single-slot pack last
    packs.sort(key=lambda pk: (-len(pk["slots"]), -pk["w"]))
    out = []
    for pk in packs:
        offs = []
        o = 0
        for j in pk["slots"]:
            offs.append((j, o, Wc[j], Lmin[j]))
            o += Wc[j]
        out.append({"Wsum": o, "Wpad": _pad16(o), "WB": _pad16(o) + 8, "slots": offs})
    return out


def build_program(packs_key, use_fp8=True):
    import concourse.bass as bass
    import concourse.bacc as bacc
    import concourse.mybir as mybir
    from concourse.tile import TileContext

    f32 = mybir.dt.float32
    cdt = mybir.dt.bfloat16
    wdt = mybir.dt.float8e4 if use_fp8 else cdt  # conv weights + F tiles
    AX = mybir.AxisListType
    ADD = mybir.AluOpType.add

    packs = [
        {"Wsum": Wsum, "Wpad": Wpad, "WB": WB, "slots": list(slots)}
        for (Wsum, Wpad, WB, slots) in packs_key
    ]
    npk = len(packs)
    Stot = sum(4 * pk["Wpad"] for pk in packs)
    foff = []
    o = 0
    for pk in packs:
        foff.append(o)
        o += 4 * pk["Wpad"]
    mo = []
    o = 0
    for pk in packs:
        mo.append(o)
        o += pk["WB"]
    ONESOFF = o
    MTOT = o + P  # trailing 128 cols of 1.0: K=1 lhsT for tensor-side masks
    WB0 = packs[0]["WB"]

    nc = bacc.Bacc()
    encF = nc.declare_dram_parameter("encF", [P, Stot], wdt, isOutput=False)
    # -1e30 at invalid mask positions, replicated on all 128 partitions
    msk = nc.declare_dram_parameter("msk", [P, MTOT], cdt, isOutput=False)
    wconv = nc.declare_dram_parameter("wconv", [P, 60 * P], wdt, isOutput=False)
    # cbias*WSCALE [:, :5] ++ fc1b (col 5) in one f32 tensor
    fcon = nc.declare_dram_parameter("fcon", [P, 7], f32, isOutput=False)
    # fc1w/WSCALE tiles ++ fc2w (col 500) in one bf16 tensor
    fcw = nc.declare_dram_parameter("fcw", [P, 5 * 100 + 1], cdt, isOutput=False)
    out = nc.declare_dram_parameter("out", [1, NSLOT], f32, isOutput=True)

    with TileContext(nc) as tc:
        with (
            tc.tile_pool(name="const", bufs=1) as constp,
            tc.tile_pool(name="pspool", bufs=8, space="PSUM") as pspool,
        ):
            wsbs = {
                name: constp.tile([P, nb * P], wdt, tag=name, name=name)
                for name, (_, i0, nb) in {v[0]: (0, v[1], v[2]) for v in _WTILES.values()}.items()
            }
            fts = [
                constp.tile([P, 4 * pk["Wpad"]], wdt, tag=f"ft{pi}", name=f"ft{pi}")
                for pi, pk in enumerate(packs)
            ]
            mska = constp.tile([P, WB0], cdt, tag="mska", name="mska")
            mskb = (
                constp.tile([P, MTOT - WB0], cdt, tag="mskb", name="mskb")
                if MTOT > WB0
                else None
            )
            fcon_sb = constp.tile([P, 7], f32, tag="fcon", name="fcon")
            fcw_sb = constp.tile([P, 5 * 100 + 1], cdt, tag="fcw", name="fcw")

            def load_wt(name):
                _, i0, nb = next(v for v in _WTILES.values() if v[0] == name)
                nc.sync.dma_start(
                    out=wsbs[name][:], in_=wconv[:, i0 * P : (i0 + nb) * P]
                )

            def load_f(eng, pi):
                eng.dma_start(
                    out=fts[pi][:],
                    in_=encF[:, foff[pi] : foff[pi] + 4 * packs[pi]["Wpad"]],
                )

            def load_wt_s(name):
                _, i0, nb = next(v for v in _WTILES.values() if v[0] == name)
                nc.scalar.dma_start(
                    out=wsbs[name][:], in_=wconv[:, i0 * P : (i0 + nb) * P]
                )

            # sync (SP) ring: first pack + w1/w45 weights, then odd packs
            load_wt("w1")
            if npk > 0:
                load_f(nc.sync, 0)
            load_wt("w45")
            for pi in range(1, npk, 2):
                load_f(nc.sync, pi)
            # scalar (ACT) ring: early w2/w3 weights + pack-0 mask, mask blob
            load_wt_s("w2")
            nc.scalar.dma_start(out=mska[:], in_=msk[:, 0:WB0])
            load_wt_s("w3")
            if npk > 2:
                load_f(nc.scalar, 2)
            if mskb is not None:
                nc.scalar.dma_start(out=mskb[:], in_=msk[:, WB0:MTOT])
            for pi in range(4, npk, 2):
                load_f(nc.scalar, pi)
            nc.scalar.dma_start(out=fcon_sb[:], in_=fcon[:])
            nc.scalar.dma_start(out=fcw_sb[:], in_=fcw[:])

            cb_sb = fcon_sb[:, 0:FS]
            fc1b_sb = fcon_sb[:100, FS : FS + 1]
            fc1w_sb = fcw_sb[:, 0 : 5 * 100]
            fc2w_sb = fcw_sb[:100, 5 * 100 : 5 * 100 + 1]

            # p-state warmup: dummy matmuls with no DMA deps run during the
            # input-DMA wait and ramp the PE clock before the real stream
            scratch = constp.tile([P, 1280], wdt, tag="scratch", name="scratch")
            nc.gpsimd.memzero(scratch[:])
            wps = pspool.tile([P, 512], f32, tag="ps", name="warmup")
            for _ in range(6):
                nc.tensor.matmul(
                    wps[:],
                    scratch[:, 0:256].rearrange("p (k m) -> p k m", k=2),
                    scratch[:, 256:1280].rearrange("p (k w) -> p k w", k=2),
                    start=True,
                    stop=True,
                    perf_mode=mybir.MatmulPerfMode.DoubleRow,
                ) if use_fp8 else nc.tensor.matmul(
                    wps[:],
                    scratch[:, 0:P],
                    scratch[:, 256:768],
                    start=True,
                    stop=True,
                )

            pools = [
                constp.tile([P, NSLOT], f32, tag=f"pool{w}", name=f"pool{w}")
                for w in range(1, FS + 1)
            ]
            poolsr = [
                constp.tile([P, NSLOT], cdt, tag=f"poolr{w}", name=f"poolr{w}")
                for w in range(1, FS + 1)
            ]

            mask_eng = nc.gpsimd if MASK_ENGINE == "gpsimd" else nc.vector
            psf1 = pspool.tile([100, NSLOT], f32, tag="ps", name="psf1")

            def conv_w(pi, w):
                pk = packs[pi]
                Wsum, Wpad = pk["Wsum"], pk["Wpad"]
                nslots = len(pk["slots"])
                Npack = Wsum - w + 1
                ps = pspool.tile([P, Npack], f32, tag="ps", name=f"ps{pi}w{w}")
                tname = _WTILES[w][0]
                wt = wsbs[tname]
                nb = _WTILES[w][2]
                if use_fp8:
                    pairs = [(dw, k0) for dw in range(w) for k0 in (0, 2)]
                else:
                    pairs = [(dw, k) for dw in range(w) for k in range(4)]
                # bf16 K=1 matmuls inside fp8 DoubleRow groups force PE mode
                # switches (~4.5us total measured) — keep masks on vector
                tmask = False
                mt, mofs = (mska, mo[pi]) if pi == 0 else (mskb, mo[pi] - WB0)
                ones_lhsT = (
                    mskb[:1, ONESOFF - WB0 : ONESOFF - WB0 + P]
                    if mskb is not None
                    else None
                )

                def bands():
                    for si, (j, off, Wcj, Lmj) in enumerate(pk["slots"]):
                        b0 = off + max(0, min(Lmj - w + 1, Wcj))
                        b1 = off + Wcj if si < nslots - 1 else Npack
                        if b0 < b1:
                            yield b0, b1

                for n, (dw, k) in enumerate(pairs):
                    idx = _WBASE[w] + dw * 4 + k
                    if use_fp8:
                        nc.tensor.matmul(
                            ps[:],
                            wt[:].rearrange("p (k m) -> p k m", k=nb)[
                                :, idx : idx + 2, :
                            ],
                            fts[pi][:].rearrange("p (k w) -> p k w", k=4)[
                                :, k : k + 2, dw : dw + Npack
                            ],
                            start=(n == 0),
                            stop=(n == len(pairs) - 1),
                            perf_mode=mybir.MatmulPerfMode.DoubleRow,
                        )
                        if n == 0 and tmask:
                            for b0, b1 in bands():
                                nc.tensor.matmul(
                                    ps[:, b0:b1],
                                    ones_lhsT,
                                    mt[:1, mofs + b0 + w : mofs + b1 + w],
                                    start=False,
                                    stop=False,
                                    tile_position=(0, 0),
                                )
                    else:
                        nc.tensor.matmul(
                            ps[:],
                            wt[:, idx * P : (idx + 1) * P],
                            fts[pi][:, k * Wpad + dw : k * Wpad + dw + Npack],
                            start=(n == 0),
                            stop=(n == len(pairs) - 1),
                        )
                if not tmask:
                    for b0, b1 in bands():
                        mask_eng.tensor_tensor(
                            ps[:, b0:b1],
                            ps[:, b0:b1],
                            mt[:, mofs + b0 + w : mofs + b1 + w],
                            ADD,
                        )
                for j, off, Wcj, Lmj in pk["slots"]:
                    nc.vector.reduce_max(
                        pools[w - 1][:, j : j + 1],
                        ps[:, off : off + Wcj - w + 1],
                        axis=AX.X,
                    )

            for pi in range(npk - 1):
                for w in range(1, FS + 1):
                    conv_w(pi, w)
            # last pack: w descending, fc1 accumulation interleaved
            def pool_ts(w):
                # gpsimd TS measured ~480ns vs vector ~180: keep on vector,
                # one engine fewer in the fc dependency chain
                nc.vector.tensor_scalar(
                    poolsr[w - 1][:],
                    pools[w - 1][:],
                    cb_sb[:, w - 1 : w],
                    0.0,
                    mybir.AluOpType.add,
                    mybir.AluOpType.max,
                )

            def fc1_mm(w, first, last):
                nc.tensor.matmul(
                    psf1[:],
                    fc1w_sb[:, (w - 1) * 100 : w * 100],
                    poolsr[w - 1][:],
                    start=first,
                    stop=last,
                )

            # last pack runs w ASCENDING (conv blocks grow toward the end) and
            # each fc1 matmul is emitted two blocks late, so the mask+reduce+
            # TS chain of its w hides under larger and larger conv streams
            for w in range(1, FS + 1):
                conv_w(npk - 1, w)
                pool_ts(w)
                if w - 2 >= 1:
                    fc1_mm(w - 2, first=(w - 2 == 1), last=False)
            fc1_mm(FS - 1, first=False, last=False)
            fc1_mm(FS, first=False, last=True)

            fc1_sb = constp.tile([100, NSLOT], cdt, tag="fc1o")
            nc.vector.tensor_scalar(
                fc1_sb[:], psf1[:], fc1b_sb, None, mybir.AluOpType.add
            )
            psf2 = pspool.tile([1, NSLOT], f32, tag="ps", name="psf2")
            nc.tensor.matmul(psf2[:], fc2w_sb, fc1_sb[:], start=True, stop=True)
            out_sb = constp.tile([1, NSLOT], f32, tag="outsb")
            nc.vector.tensor_scalar(
                out_sb[:], psf2[:], 0.0, None, mybir.AluOpType.add
            )
            nc.sync.dma_start(out=out[:], in_=out_sb[:])

    nc.compile()
    return nc


def prepare(encoder_output, lengths, conv_ws, conv_bs, fc1_w, fc1_b, fc2_w, fc2_b,
            use_fp8=None):
    """Host-side prep: sample assignment, per-core data tables, program build.

    Returns (nc, in_maps, assignment, fc2b) where assignment[c][j] = sample.
    """
    if use_fp8 is None:
        use_fp8 = USE_FP8
    enc = np.ascontiguousarray(np.asarray(encoder_output, dtype=np.float32))
    lens = np.asarray(lengths).astype(np.int64)
    assert enc.shape == (T, B, H)
    assert lens.shape == (B,)

    cdt = ml_dtypes.bfloat16
    wdt = ml_dtypes.float8_e4m3 if use_fp8 else cdt
    wscale = np.float32(WSCALE if use_fp8 else 1.0)

    # effective lengths (torch zero-pads width to >= filter_size)
    eff = np.maximum(lens, FS)

    # sort desc by effective length; slot j <- ranks [8j, 8j+8)
    ranks = np.argsort(-eff, kind="stable")
    assignment = [[int(ranks[8 * j + c]) for j in range(NSLOT)] for c in range(NCORES)]
    Wc = tuple(int(eff[ranks[8 * j]]) for j in range(NSLOT))
    Lmin = tuple(int(eff[ranks[8 * j + NCORES - 1]]) for j in range(NSLOT))
    packs = make_packs(Wc, Lmin)
    packs_key = tuple(
        (pk["Wsum"], pk["Wpad"], pk["WB"], tuple(pk["slots"])) for pk in packs
    )

    encT = enc.transpose(1, 0, 2)  # [B, T, H], sample-major views

    Stot = sum(4 * pk["Wpad"] for pk in packs)
    foff = []
    o = 0
    for pk in packs:
        foff.append(o)
        o += 4 * pk["Wpad"]
    mo = []
    o = 0
    for pk in packs:
        mo.append(o)
        o += pk["WB"]
    ONESOFF = o
    MTOT = o + P

    in_maps = []
    for c in range(NCORES):
        encF_c = np.zeros((P, Stot), dtype=np.float32)
        mrow_all = np.zeros(MTOT, dtype=np.float32)  # [.., ONESOFF): masks; rest: 1.0
        for pi, pk in enumerate(packs):
            Wpad = pk["Wpad"]
            blk = encF_c[:, foff[pi] : foff[pi] + 4 * Wpad].reshape(P, 4, Wpad)
            mrow = mrow_all[mo[pi] : mo[pi] + pk["WB"]]
            nslots = len(pk["slots"])
            Les = []
            for si, (j, off, Wcj, Lmj) in enumerate(pk["slots"]):
                b = assignment[c][j]
                L = int(lens[b])
                Le = int(eff[b])
                Les.append(Le)
                flat = encT[b].reshape(-1)[: H * L]
                blk[:, :, off : off + L] = flat.reshape(P, 4, L)
                # mask row: 1 (invalid) where u > off + Le, u in this slot's
                # span [off, off+Wcj) (last slot: through WB)
                hi = off + Wcj if si < nslots - 1 else pk["WB"]
                u = np.arange(off, hi)
                mrow[off:hi] = (u > off + Le).astype(np.float32)
            for si, (j, off, Wcj, Lmj) in enumerate(pk["slots"]):
                if si < nslots - 1:
                    # u in [off+Wcj, off+Wcj+FS) is read only by THIS slot's
                    # bands: u == off+Wcj iff this core's sample is short;
                    # u > off+Wcj is a boundary-crossing window, always bad
                    mrow[off + Wcj] = 1.0 if Les[si] < Wcj else 0.0
                    mrow[off + Wcj + 1 : min(off + Wcj + FS, pk["WB"])] = 1.0
        mrow_all *= np.float32(-1e30)
        mrow_all[ONESOFF:] = 1.0
        in_maps.append(
            {
                "encF": encF_c.astype(wdt),
                "msk": np.broadcast_to(
                    mrow_all.astype(cdt)[None, :], (P, MTOT)
                ).copy(),
            }
        )

    # weights, shared across cores
    wconv = np.empty((P, 60 * P), dtype=np.float32)
    hsel = np.arange(P)[:, None] * 4  # [128,1]
    for w in range(1, FS + 1):
        Ww = np.asarray(conv_ws[w - 1], dtype=np.float32)  # [NF, 1, H, w]
        for dw in range(w):
            i = _pair_index(w, dw)
            for k in range(4):
                # lhsT[p, f] = Ww[f, 0, 4p+k, dw] * wscale
                wconv[:, (i * 4 + k) * P : (i * 4 + k + 1) * P] = (
                    Ww[:, 0, (hsel + k).ravel(), dw].T * wscale
                )
    fcon = np.zeros((P, 7), dtype=np.float32)
    fcon[:, 0:FS] = (
        np.stack([np.asarray(b, dtype=np.float32) for b in conv_bs], axis=1) * wscale
    )
    fcon[:100, FS] = np.asarray(fc1_b, dtype=np.float32)
    fcw_host = np.zeros((P, 5 * 100 + 1), dtype=np.float32)
    fc1_w = np.asarray(fc1_w, dtype=np.float32) / wscale  # [100, 640], dequant
    for k in range(5):
        fcw_host[:, k * 100 : (k + 1) * 100] = fc1_w[:, k * P : (k + 1) * P].T
    fcw_host[:100, 5 * 100] = np.asarray(fc2_w, dtype=np.float32).reshape(-1)
    shared = {
        "wconv": wconv.astype(wdt),
        "fcon": fcon,
        "fcw": fcw_host.astype(cdt),
    }
    for m in in_maps:
        m.update(shared)

    key = (packs_key, use_fp8, MASK_ENGINE)
    if key not in _PROGRAM_CACHE:
        _PROGRAM_CACHE[key] = build_program(packs_key, use_fp8)
    nc = _PROGRAM_CACHE[key]
    fc2b = float(np.asarray(fc2_b, dtype=np.float32).reshape(-1)[0])
    return nc, in_maps, assignment, fc2b


def _ensure_ntff_hook():
    """Install the axon NTFF profile hook if the image's antenv lacks it."""
    import types

    try:
        from antenv.axon_hooks import get_axon_ntff_profile_hook  # noqa: F401
        return True
    except ImportError:
        pass
    try:
        import antenv
        from trn_agent_boot.trn_boot import _ntff_profile_via_ctypes

        hook = _ntff_profile_via_ctypes("/opt/axon/libaxon_pjrt.so")
        mod = types.ModuleType("antenv.axon_hooks")
        _state = {"hook": hook}
        mod.get_axon_ntff_profile_hook = lambda: _state["hook"]
        mod.set_axon_ntff_profile_hook = lambda h: _state.update(hook=h)
        sys.modules["antenv.axon_hooks"] = mod
        antenv.axon_hooks = mod
        return hook is not None
    except Exception as e:  # pragma: no cover
        print(f"ntff hook install failed: {e}", file=sys.stderr)
        return False


def kernel(encoder_output, lengths,
           conv_w1, conv_b1, conv_w2, conv_b2, conv_w3, conv_b3,
           conv_w4, conv_b4, conv_w5, conv_b5,
           fc1_w, fc1_b, fc2_w, fc2_b):
    global LAST_EXEC_NS, LAST_RESULTS
    from concourse.bass_utils import run_bass_kernel_spmd

    conv_ws = [conv_w1, conv_w2, conv_w3, conv_w4, conv_w5]
    conv_bs = [conv_b1, conv_b2, conv_b3, conv_b4, conv_b5]
    nc, in_maps, assignment, fc2b = prepare(
        encoder_output, lengths, conv_ws, conv_bs, fc1_w, fc1_b, fc2_w, fc2_b
    )

    trace = bool(int(os.environ.get("KERNEL_TRACE", "0")))
    if trace:
        trace = _ensure_ntff_hook()
    res = run_bass_kernel_spmd(nc, in_maps, list(range(NCORES)), trace=trace)
    LAST_RESULTS = res
    LAST_EXEC_NS = getattr(res, "exec_time_ns", None)

    out_full = np.empty((B, 1, 1), dtype=np.float32)
    for c in range(NCORES):
        logits = np.asarray(res.results[c]["out"]).reshape(NSLOT).astype(np.float64)
        probs = 1.0 / (1.0 + np.exp(-(logits + fc2b)))
        for j in range(NSLOT):
            out_full[assignment[c][j], 0, 0] = np.float32(probs[j])
    return out_full
